# revision 8
# baseline (speedup 1.0000x reference)
"""Fused dual-stream sliding-window attention for Trainium2 (Bass/Tile).

The reference computes two banded softmax streams (s: 0<=i-j<W, c: W<=i-j<2W)
and merges them via LSE. Over disjoint key sets that merge is exactly one
softmax over the union band 0 <= i-j < 2W (W=256), so we compute a single
fused banded attention.

Layout strategy (per (batch, head) pair, sharded 4 pairs/core x 8 cores):
  - host pre-transposes Q, K to [D=128, S] so the kernel never transposes
  - per query block b (256 rows), context = key blocks [b-2, b-1, b]
    = 6 chunks of 128 keys, computed in S^T orientation [ck, q]:
        S^T_chunk = matmul(lhsT=K^T[:, chunk], rhs=Q^T[:, block])   # [128, 256]
        p^T = exp(S^T * D^-0.5)        (ACT, scale fused, 2 batched ops)
        p^T *= triangle mask           (DVE, one batched op)
        out^T accum: matmul(lhsT=p^T[:, half], rhs=V_aug[chunk])    # [128, 129]
    V_aug has a ones column at 128 so psum col 128 accumulates the softmax
    denominator.
  - normalize with DVE reciprocal + one broadcasted tensor_tensor, DMA out.

All matmuls run in float32r (FP22 mantissa truncation, full PE rate) with
fp32 accumulation.  Chunks are grouped into two multi-bank PSUM tiles
(A = chunks 0,1,4,5 -> the masked ones, B = chunks 2,3) so exp and the mask
multiply run as single strided ops.  A short burst of dummy matmuls at kernel
start keeps the PE busy through the initial DMA so the HAM clock-gate is warm
when real work begins.
"""

import numpy as np

import concourse.bass as bass
from concourse import bacc
import concourse.mybir as mybir
import concourse.tile as tile
from concourse.bass_utils import run_bass_kernel_spmd

B, S, H, D = 2, 2048, 16, 128
WIN = 256
N_CORES = 8
PAIRS = (B * H) // N_CORES          # 4 (batch, head) pairs per core
NB = S // WIN                       # 8 query blocks per sequence
NG = S // 128                       # 16 key subtiles of 128 per sequence
SCALE = float(D) ** -0.5
F32 = mybir.dt.float32
F32R = mybir.dt.float32r
EXP = mybir.ActivationFunctionType.Exp

# chunk -> (tile, slot): A holds the maskable chunks in mask order, B the rest
A_SLOT = {0: 0, 1: 1, 4: 2, 5: 3}
B_SLOT = {2: 0, 3: 1}
# (chunk, half) subtiles that are entirely masked out -> skip their PV matmul
EMPTY_SUBTILES = {(0, 1), (5, 0)}
VW = 136          # v tile slot stride (128 data + 1 ones + pad to 32B align)
N_WARMUP = 110    # dummy matmuls covering the initial DMA to keep HAM warm


def build_masks() -> np.ndarray:
    """0/1 triangle masks in the S^T layout: partition p = key-in-chunk,
    free f = query-in-block.  Valid band: f - p in [128*c - 512, 128*c - 1].
    Slot order matches A_SLOT: chunks 0, 1, 4, 5."""
    p = np.arange(128)[:, None]
    f = np.arange(256)[None, :]
    m = np.zeros((128, 4, 256), np.float32)
    m[:, 0, :] = f < p            # chunk 0
    m[:, 1, :] = f < p + 128      # chunk 1
    m[:, 2, :] = f >= p           # chunk 4
    m[:, 3, :] = f >= p + 128     # chunk 5
    return m


def chunks_for_block(b: int) -> list[int]:
    # chunk c of query block b reads key subtile g = 2b - 4 + c; g must be >= 0
    return list(range(max(0, 4 - 2 * b), 6))


def build_program() -> bacc.Bacc:
    nc = bacc.Bacc("TRN2", target_bir_lowering=False, debug=False)

    qt = nc.dram_tensor("qt", [PAIRS, 128, S], F32R, kind="ExternalInput").ap()
    kt = nc.dram_tensor("kt", [PAIRS, 128, S], F32R, kind="ExternalInput").ap()
    vv = nc.dram_tensor("v", [PAIRS, S, 128], F32R, kind="ExternalInput").ap()
    mk = nc.dram_tensor("masks", [128, 4, 256], F32, kind="ExternalInput").ap()
    out = nc.dram_tensor("out", [PAIRS, S, 128], F32, kind="ExternalOutput").ap()

    with tile.TileContext(nc) as tc:
        with (
            tc.tile_pool(name="const", bufs=1) as const_pool,
            tc.tile_pool(name="qtp", bufs=2 * NB) as qt_pool,
            tc.tile_pool(name="ktp", bufs=8) as kt_pool,
            tc.tile_pool(name="vp", bufs=8) as v_pool,
            tc.tile_pool(name="ptA", bufs=3) as ptA_pool,
            tc.tile_pool(name="ptB", bufs=3) as ptB_pool,
            tc.tile_pool(name="stA", bufs=1, space="PSUM") as stA_pool,
            tc.tile_pool(name="stB", bufs=1, space="PSUM") as stB_pool,
            tc.tile_pool(name="pv", bufs=2, space="PSUM") as pv_pool,
            tc.tile_pool(name="outp", bufs=4) as out_pool,
            tc.tile_pool(name="rcp", bufs=4) as rcp_pool,
        ):
            mask_sb = const_pool.tile([128, 4, 256], F32)
            nc.sync.dma_start(mask_sb[:], mk[:])

            # PE warm-up: harmless matmuls on a zeroed tile while the first
            # pair's DMAs land, so HAM reaches K=8/8 before real work.
            warm = const_pool.tile([128, 128], F32R)
            nc.gpsimd.memset(warm[:].bitcast(F32), 0.0)
            wpsum = pv_pool.tile([128, 2, VW], F32, tag="pv")
            for _ in range(N_WARMUP):
                nc.tensor.matmul(wpsum[:, 0, 0:64], lhsT=warm[:, 0:128], rhs=warm[:, 0:64],
                                 start=True, stop=True)

            for pair in range(PAIRS):
                # inputs split into pieces ordered by first use so compute
                # starts as soon as the early pieces land
                qt_t, kt_t, v_t = [], [], []

                def load_piece(j, pair=pair, kt_t=kt_t, v_t=v_t):
                    k_tile = kt_pool.tile([128, 512], F32R)
                    nc.sync.dma_start(k_tile[:],
                                      kt[pair, :, j * 512:(j + 1) * 512])
                    kt_t.append(k_tile)
                    vt = v_pool.tile([128, 4, VW], F32R)
                    nc.gpsimd.memset(vt[:, :, 128:130].bitcast(F32), 1.0)
                    nc.sync.dma_start(
                        vt[:, :, 0:128],
                        vv[pair, j * 512:(j + 1) * 512, :].rearrange(
                            "(g p) d -> p g d", p=128),
                    )
                    v_t.append(vt)

                def load_q(b, pair=pair, qt_t=qt_t):
                    q_tile = qt_pool.tile([128, 256], F32R)
                    nc.sync.dma_start(q_tile[:],
                                      qt[pair, :, b * 256:(b + 1) * 256])
                    qt_t.append(q_tile)

                load_q(0)
                load_piece(0)
                load_q(1)
                load_q(2)
                load_piece(1)
                load_q(3)
                load_q(4)
                load_piece(2)
                load_q(5)
                load_q(6)
                load_piece(3)
                load_q(7)

                for b in range(NB):
                    cs = chunks_for_block(b)
                    a_lo = A_SLOT[min(c for c in cs if c in A_SLOT)]
                    has_b = 2 in cs or 3 in cs

                    stA = stA_pool.tile([128, 4, 512], F32)
                    stB = None
                    if has_b:
                        stB = stB_pool.tile([128, 2, 512], F32, tag="stB")
                    for c in cs:
                        g = 2 * b - 4 + c
                        if c in A_SLOT:
                            dst = stA[:, A_SLOT[c], 0:256]
                        else:
                            dst = stB[:, B_SLOT[c], 0:256]
                        nc.tensor.matmul(
                            dst,
                            lhsT=kt_t[g // 4][:, (g % 4) * 128:(g % 4 + 1) * 128],
                            rhs=qt_t[b][:],
                            start=True, stop=True,
                        )

                    pTA = ptA_pool.tile([128, 4, 256], F32R)
                    nc.scalar.activation(pTA[:, a_lo:4, :], stA[:, a_lo:4, 0:256],
                                         EXP, scale=SCALE)
                    nc.vector.tensor_mul(pTA[:, a_lo:4, :], pTA[:, a_lo:4, :],
                                         mask_sb[:, a_lo:4, :])
                    pTB = None
                    if has_b:
                        pTB = ptB_pool.tile([128, 2, 256], F32R)
                        nc.scalar.activation(pTB[:], stB[:, :, 0:256],
                                             EXP, scale=SCALE)

                    pv = pv_pool.tile([128, 2, VW], F32, tag="pv")
                    for h in (0, 1):
                        mms = [c for c in cs if (c, h) not in EMPTY_SUBTILES]
                        for i, c in enumerate(mms):
                            g = 2 * b - 4 + c
                            if c in A_SLOT:
                                lhsT = pTA[:, A_SLOT[c], h * 128:(h + 1) * 128]
                            else:
                                lhsT = pTB[:, B_SLOT[c], h * 128:(h + 1) * 128]
                            nc.tensor.matmul(
                                pv[:, h, 0:130],
                                lhsT=lhsT,
                                rhs=v_t[g // 4][:, g % 4, 0:130],
                                start=(i == 0), stop=(i == len(mms) - 1),
                            )

                    recip = rcp_pool.tile([128, 2], F32)
                    nc.vector.reciprocal(recip[:], pv[:, :, 128])
                    ot = out_pool.tile([128, 2, 128], F32)
                    nc.vector.tensor_mul(
                        ot[:], pv[:, :, 0:128],
                        recip[:].unsqueeze(2).broadcast_to([128, 2, 128]),
                    )
                    nc.sync.dma_start(
                        out[pair, b * 256:(b + 1) * 256, :].rearrange(
                            "(h p) d -> p h d", h=2),
                        ot[:],
                    )
    nc.compile()
    return nc


_CACHE: dict = {}


def _get_program() -> bacc.Bacc:
    if "nc" not in _CACHE:
        _CACHE["nc"] = build_program()
    return _CACHE["nc"]


def make_in_maps(query, key, value):
    """Shard + pre-transpose full [B,S,H,D] inputs into per-core input maps."""
    qt_all = np.ascontiguousarray(query.transpose(0, 2, 3, 1))  # [B,H,D,S]
    kt_all = np.ascontiguousarray(key.transpose(0, 2, 3, 1))
    v_all = np.ascontiguousarray(value.transpose(0, 2, 1, 3))   # [B,H,S,D]
    masks = build_masks()
    in_maps = []
    for c in range(N_CORES):
        idx = [divmod(c * PAIRS + i, H) for i in range(PAIRS)]
        in_maps.append({
            "qt": np.stack([qt_all[b, h] for b, h in idx]),
            "kt": np.stack([kt_all[b, h] for b, h in idx]),
            "v": np.stack([v_all[b, h] for b, h in idx]),
            "masks": masks,
        })
    return in_maps


def gather_output(results) -> np.ndarray:
    out = np.empty((B, S, H, D), np.float32)
    for c in range(N_CORES):
        o = results[c]["out"]
        for i in range(PAIRS):
            b, h = divmod(c * PAIRS + i, H)
            out[b, :, h, :] = o[i]
    return out


def run(query, key, value, trace: bool = False):
    nc = _get_program()
    in_maps = make_in_maps(query, key, value)
    res = run_bass_kernel_spmd(nc, in_maps, core_ids=list(range(N_CORES)),
                               trace=trace)
    return gather_output(res.results), res


def kernel(query, key, value):
    out, _ = run(query, key, value)
    return out


# revision 9
# speedup vs baseline: 1.3907x; 1.3907x over previous
"""Fused dual-stream sliding-window attention for Trainium2 (Bass/Tile).

The reference computes two banded softmax streams (s: 0<=i-j<W, c: W<=i-j<2W)
and merges them via LSE. Over disjoint key sets that merge is exactly one
softmax over the union band 0 <= i-j < 2W (W=256), so we compute a single
fused banded attention.

Layout strategy (per (batch, head) pair, sharded 4 pairs/core x 8 cores):
  - host pre-transposes Q, K to [D=128, S] so the kernel never transposes
  - per query block b (256 rows), context = key blocks [b-2, b-1, b]
    = 6 chunks of 128 keys, computed in S^T orientation [ck, q]:
        S^T_chunk = matmul(lhsT=K^T[:, chunk], rhs=Q^T[:, block])   # [128, 256]
        p^T = exp(S^T * D^-0.5)        (ACT, scale fused, 2 batched ops)
        p^T *= triangle mask           (DVE, one batched op)
        out^T accum: matmul(lhsT=p^T[:, half], rhs=V_aug[chunk])    # [128, 129]
    V_aug has a ones column at 128 so psum col 128 accumulates the softmax
    denominator.
  - normalize with DVE reciprocal + one broadcasted tensor_tensor, DMA out.

All matmuls run in float32r (FP22 mantissa truncation, full PE rate) with
fp32 accumulation.  Chunks are grouped into two multi-bank PSUM tiles
(A = chunks 0,1,4,5 -> the masked ones, B = chunks 2,3) so exp and the mask
multiply run as single strided ops.  A short burst of dummy matmuls at kernel
start keeps the PE busy through the initial DMA so the HAM clock-gate is warm
when real work begins.
"""

import numpy as np

import concourse.bass as bass
from concourse import bacc
import concourse.mybir as mybir
import concourse.tile as tile
from concourse.bass_utils import run_bass_kernel_spmd

B, S, H, D = 2, 2048, 16, 128
WIN = 256
N_CORES = 8
PAIRS = (B * H) // N_CORES          # 4 (batch, head) pairs per core
NB = S // WIN                       # 8 query blocks per sequence
NG = S // 128                       # 16 key subtiles of 128 per sequence
SCALE = float(D) ** -0.5
F32 = mybir.dt.float32
F32R = mybir.dt.float32r
EXP = mybir.ActivationFunctionType.Exp

# chunk -> (tile, slot): A holds the maskable chunks in mask order, B the rest
A_SLOT = {0: 0, 1: 1, 4: 2, 5: 3}
B_SLOT = {2: 0, 3: 1}
# (chunk, half) subtiles that are entirely masked out -> skip their PV matmul
EMPTY_SUBTILES = {(0, 1), (5, 0)}
VW = 136          # v tile slot stride (128 data + 1 ones + pad to 32B align)
N_WARMUP = 48     # dummy matmuls covering the initial DMA to keep HAM warm


def build_masks() -> np.ndarray:
    """0/1 triangle masks in the S^T layout: partition p = key-in-chunk,
    free f = query-in-block.  Valid band: f - p in [128*c - 512, 128*c - 1].
    Slot order matches A_SLOT: chunks 0, 1, 4, 5."""
    p = np.arange(128)[:, None]
    f = np.arange(256)[None, :]
    m = np.zeros((128, 4, 256), np.float32)
    m[:, 0, :] = f < p            # chunk 0
    m[:, 1, :] = f < p + 128      # chunk 1
    m[:, 2, :] = f >= p           # chunk 4
    m[:, 3, :] = f >= p + 128     # chunk 5
    return m


def chunks_for_block(b: int) -> list[int]:
    # chunk c of query block b reads key subtile g = 2b - 4 + c; g must be >= 0
    return list(range(max(0, 4 - 2 * b), 6))


def build_program() -> bacc.Bacc:
    nc = bacc.Bacc("TRN2", target_bir_lowering=False, debug=False)

    qt = nc.dram_tensor("qt", [PAIRS, 128, S], F32R, kind="ExternalInput").ap()
    kt = nc.dram_tensor("kt", [PAIRS, 128, S], F32R, kind="ExternalInput").ap()
    vv = nc.dram_tensor("v", [PAIRS, S, 130], F32R, kind="ExternalInput").ap()
    mk = nc.dram_tensor("masks", [128, 4, 256], F32, kind="ExternalInput").ap()
    out = nc.dram_tensor("out", [PAIRS, S, 128], F32, kind="ExternalOutput").ap()

    with tile.TileContext(nc) as tc:
        with (
            tc.tile_pool(name="const", bufs=1) as const_pool,
            tc.tile_pool(name="qtp", bufs=2 * NB) as qt_pool,
            tc.tile_pool(name="ktp", bufs=8) as kt_pool,
            tc.tile_pool(name="vp", bufs=8) as v_pool,
            tc.tile_pool(name="ptA", bufs=3) as ptA_pool,
            tc.tile_pool(name="ptB", bufs=3) as ptB_pool,
            tc.tile_pool(name="stA", bufs=2, space="PSUM") as stA_pool,
            tc.tile_pool(name="stB", bufs=2, space="PSUM") as stB_pool,
            tc.tile_pool(name="pv", bufs=2, space="PSUM") as pv_pool,
            tc.tile_pool(name="outp", bufs=4) as out_pool,
            tc.tile_pool(name="rcp", bufs=4) as rcp_pool,
        ):
            mask_sb = const_pool.tile([128, 4, 256], F32)
            nc.sync.dma_start(mask_sb[:], mk[:])

            # PE warm-up: harmless matmuls on a zeroed tile while the first
            # pair's DMAs land, so HAM reaches K=8/8 before real work.
            warm = const_pool.tile([128, 128], mybir.dt.bfloat16)
            nc.gpsimd.memset(warm[:], 0.0)
            wpsum = pv_pool.tile([128, 2, VW], F32, tag="pv")
            for _ in range(N_WARMUP):
                nc.tensor.matmul(wpsum[:, 0, 0:32], lhsT=warm[:, 0:128],
                                 rhs=warm[:, 0:32], start=True, stop=True)

            for pair in range(PAIRS):
                # inputs split into pieces ordered by first use so compute
                # starts as soon as the early pieces land
                qt_t, kt_t, v_t = [], [], []

                def load_piece(j, pair=pair, kt_t=kt_t, v_t=v_t):
                    k_tile = kt_pool.tile([128, 512], F32R)
                    nc.sync.dma_start(k_tile[:],
                                      kt[pair, :, j * 512:(j + 1) * 512])
                    kt_t.append(k_tile)
                    vt = v_pool.tile([128, 4, VW], F32R)
                    nc.sync.dma_start(
                        vt[:, :, 0:130],
                        vv[pair, j * 512:(j + 1) * 512, :].rearrange(
                            "(g p) d -> p g d", p=128),
                    )
                    v_t.append(vt)

                def load_q(b, pair=pair, qt_t=qt_t):
                    q_tile = qt_pool.tile([128, 256], F32R)
                    nc.sync.dma_start(q_tile[:],
                                      qt[pair, :, b * 256:(b + 1) * 256])
                    qt_t.append(q_tile)

                load_q(0)
                load_piece(0)
                load_q(1)
                load_q(2)
                load_piece(1)
                load_q(3)
                load_q(4)
                load_piece(2)
                load_q(5)
                load_q(6)
                load_piece(3)
                load_q(7)

                for b in range(NB):
                    cs = chunks_for_block(b)
                    a_lo = A_SLOT[min(c for c in cs if c in A_SLOT)]
                    has_b = 2 in cs or 3 in cs

                    stA = stA_pool.tile([128, 4, 256], F32)
                    stB = None
                    if has_b:
                        stB = stB_pool.tile([128, 2, 256], F32, tag="stB")
                    for c in cs:
                        g = 2 * b - 4 + c
                        if c in A_SLOT:
                            dst = stA[:, A_SLOT[c], :]
                        else:
                            dst = stB[:, B_SLOT[c], :]
                        nc.tensor.matmul(
                            dst,
                            lhsT=kt_t[g // 4][:, (g % 4) * 128:(g % 4 + 1) * 128],
                            rhs=qt_t[b][:],
                            start=True, stop=True,
                        )

                    pTA = ptA_pool.tile([128, 4, 256], F32R)
                    nc.scalar.activation(pTA[:, a_lo:4, :], stA[:, a_lo:4, :],
                                         EXP, scale=SCALE)
                    nc.vector.tensor_mul(pTA[:, a_lo:4, :], pTA[:, a_lo:4, :],
                                         mask_sb[:, a_lo:4, :])
                    pTB = None
                    if has_b:
                        pTB = ptB_pool.tile([128, 2, 256], F32R)
                        nc.scalar.activation(pTB[:], stB[:, :, :],
                                             EXP, scale=SCALE)

                    pv = pv_pool.tile([128, 2, VW], F32, tag="pv")
                    for h in (0, 1):
                        mms = [c for c in (2, 3, 0, 1, 4, 5)
                               if c in cs and (c, h) not in EMPTY_SUBTILES]
                        for i, c in enumerate(mms):
                            g = 2 * b - 4 + c
                            if c in A_SLOT:
                                lhsT = pTA[:, A_SLOT[c], h * 128:(h + 1) * 128]
                            else:
                                lhsT = pTB[:, B_SLOT[c], h * 128:(h + 1) * 128]
                            nc.tensor.matmul(
                                pv[:, h, 0:130],
                                lhsT=lhsT,
                                rhs=v_t[g // 4][:, g % 4, 0:130],
                                start=(i == 0), stop=(i == len(mms) - 1),
                            )

                    recip = rcp_pool.tile([128, 2], F32)
                    nc.vector.reciprocal(recip[:], pv[:, :, 128])
                    ot = out_pool.tile([128, 2, 128], F32)
                    nc.vector.tensor_mul(
                        ot[:], pv[:, :, 0:128],
                        recip[:].unsqueeze(2).broadcast_to([128, 2, 128]),
                    )
                    nc.sync.dma_start(
                        out[pair, b * 256:(b + 1) * 256, :].rearrange(
                            "(h p) d -> p h d", h=2),
                        ot[:],
                    )
    nc.compile()
    return nc


_CACHE: dict = {}


def _get_program() -> bacc.Bacc:
    if "nc" not in _CACHE:
        _CACHE["nc"] = build_program()
    return _CACHE["nc"]


def make_in_maps(query, key, value):
    """Shard + pre-transpose full [B,S,H,D] inputs into per-core input maps."""
    qt_all = np.ascontiguousarray(query.transpose(0, 2, 3, 1))  # [B,H,D,S]
    kt_all = np.ascontiguousarray(key.transpose(0, 2, 3, 1))
    v_all = np.empty((B, H, S, 130), np.float32)        # [B,H,S,D+2ones]
    v_all[:, :, :, 0:128] = value.transpose(0, 2, 1, 3)
    v_all[:, :, :, 128:130] = 1.0
    masks = build_masks()
    in_maps = []
    for c in range(N_CORES):
        idx = [divmod(c * PAIRS + i, H) for i in range(PAIRS)]
        in_maps.append({
            "qt": np.stack([qt_all[b, h] for b, h in idx]),
            "kt": np.stack([kt_all[b, h] for b, h in idx]),
            "v": np.stack([v_all[b, h] for b, h in idx]),
            "masks": masks,
        })
    return in_maps


def gather_output(results) -> np.ndarray:
    out = np.empty((B, S, H, D), np.float32)
    for c in range(N_CORES):
        o = results[c]["out"]
        for i in range(PAIRS):
            b, h = divmod(c * PAIRS + i, H)
            out[b, :, h, :] = o[i]
    return out


def run(query, key, value, trace: bool = False):
    nc = _get_program()
    in_maps = make_in_maps(query, key, value)
    res = run_bass_kernel_spmd(nc, in_maps, core_ids=list(range(N_CORES)),
                               trace=trace)
    return gather_output(res.results), res


def kernel(query, key, value):
    out, _ = run(query, key, value)
    return out


# revision 11
# speedup vs baseline: 2.2653x; 1.6289x over previous
"""Fused dual-stream sliding-window attention for Trainium2 (Bass/Tile).

The reference computes two banded softmax streams (s: 0<=i-j<W, c: W<=i-j<2W)
and merges them via LSE. Over disjoint key sets that merge is exactly one
softmax over the union band 0 <= i-j < 2W (W=256), so we compute a single
fused banded attention.

Layout strategy (per (batch, head) pair, sharded 4 pairs/core x 8 cores):
  - host pre-transposes Q, K to [D=128, S] (and casts to bf16) so the kernel
    never transposes
  - per query block b (256 rows), context = key blocks [b-2, b-1, b]
    = 6 chunks of 128 keys, computed in S^T orientation [ck, q]:
        S^T_chunk = matmul(lhsT=K^T[:, chunk], rhs=Q^T[:, block])   # [128, 256]
        p^T = exp(S^T * D^-0.5)        (ACT, scale fused, batched)
        p^T *= triangle mask           (DVE bf16 2x mode, batched)
        out^T accum: matmul(lhsT=p^T[:, half], rhs=V_aug[chunk])    # [128, 130]
    V_aug has ones columns at 128/129 (prefilled host-side) so psum col 128
    accumulates the softmax denominator.
  - normalize with DVE reciprocal + one broadcasted tensor_tensor, DMA out
    (fp32, via GPSIMD's SWDGE ring so stores never block input prefetch).

Matmuls run in bf16 (inputs quantized host-side) with fp32 PSUM accumulation.
The 4 maskable chunks live in one PSUM tile A with slot order [c5 c1 c4 c0],
placing the two all-masked half-tiles at the flat ends, so exp + mask are
single strided ops over the interior; chunks 2/3 (never masked) live in tile
B.  st tiles pack two 1KB chunk outputs per PSUM bank so A+B double-buffered
plus the PV accumulator fit exactly in the 8 banks.  A short burst of dummy
bf16 matmuls at kernel start keeps the PE busy through the initial DMA so the
HAM clock-gate is warm when real work begins.
"""

import ml_dtypes
import numpy as np

import concourse.bass as bass
from concourse import bacc
import concourse.mybir as mybir
import concourse.tile as tile
from concourse.bass_utils import run_bass_kernel_spmd

B, S, H, D = 2, 2048, 16, 128
WIN = 256
N_CORES = 8
PAIRS = (B * H) // N_CORES          # 4 (batch, head) pairs per core
NB = S // WIN                       # 8 query blocks per sequence
SCALE = float(D) ** -0.5
F32 = mybir.dt.float32
BF16 = mybir.dt.bfloat16
NP_BF16 = ml_dtypes.bfloat16
EXP = mybir.ActivationFunctionType.Exp

# chunk -> slot in the A (maskable) / B (never masked) PSUM tiles.  A's order
# [c5 c1 c4 c0] puts the fully-masked halves (c5 h0, c0 h1) at the flat ends.
A_SLOT = {5: 0, 1: 1, 4: 2, 0: 3}
B_SLOT = {2: 0, 3: 1}
# (chunk, half) subtiles that are entirely masked out -> skip their PV matmul
EMPTY_SUBTILES = {(0, 1), (5, 0)}
VW = 136          # v tile slot stride (128 data + 2 ones + pad)
N_WARMUP = 40     # dummy matmuls covering the initial DMA to keep HAM warm


def build_masks() -> np.ndarray:
    """0/1 triangle masks in the S^T layout: partition p = key-in-chunk,
    free f = query-in-block.  Valid band: f - p in [128*c - 512, 128*c - 1].
    Slot order matches A_SLOT: chunks 5, 1, 4, 0."""
    p = np.arange(128)[:, None]
    f = np.arange(256)[None, :]
    m = np.zeros((128, 4, 256), np.float32)
    m[:, 0, :] = f >= p + 128     # chunk 5
    m[:, 1, :] = f < p + 128      # chunk 1
    m[:, 2, :] = f >= p           # chunk 4
    m[:, 3, :] = f < p            # chunk 0
    return m.astype(NP_BF16)


def chunks_for_block(b: int) -> list[int]:
    # chunk c of query block b reads key subtile g = 2b - 4 + c; g must be >= 0
    return list(range(max(0, 4 - 2 * b), 6))


def build_program() -> bacc.Bacc:
    nc = bacc.Bacc("TRN2", target_bir_lowering=False, debug=False)

    qt = nc.dram_tensor("qt", [PAIRS, 128, S], BF16, kind="ExternalInput").ap()
    kt = nc.dram_tensor("kt", [PAIRS, 128, S], BF16, kind="ExternalInput").ap()
    vv = nc.dram_tensor("v", [PAIRS, S, 130], BF16, kind="ExternalInput").ap()
    mk = nc.dram_tensor("masks", [128, 4, 256], BF16, kind="ExternalInput").ap()
    out = nc.dram_tensor("out", [PAIRS, S, 128], F32, kind="ExternalOutput").ap()

    with tile.TileContext(nc) as tc:
        with (
            tc.tile_pool(name="const", bufs=1) as const_pool,
            tc.tile_pool(name="qtp", bufs=2 * NB) as qt_pool,
            tc.tile_pool(name="ktp", bufs=8) as kt_pool,
            tc.tile_pool(name="vp", bufs=8) as v_pool,
            tc.tile_pool(name="ptA", bufs=3) as ptA_pool,
            tc.tile_pool(name="ptB", bufs=3) as ptB_pool,
            tc.tile_pool(name="stA", bufs=2, space="PSUM") as stA_pool,
            tc.tile_pool(name="stB", bufs=2, space="PSUM") as stB_pool,
            tc.tile_pool(name="pv", bufs=2, space="PSUM") as pv_pool,
            tc.tile_pool(name="outp", bufs=4) as out_pool,
            tc.tile_pool(name="rcp", bufs=4) as rcp_pool,
        ):
            mask_sb = const_pool.tile([128, 4, 256], BF16)
            nc.sync.dma_start(mask_sb[:], mk[:])

            # PE warm-up: harmless matmuls while the first pair's DMAs land,
            # so HAM reaches K=8/8 before real work. Reads uninitialized SBUF;
            # the psum results are never read (next start=True resets).
            warm = const_pool.tile([128, 128], BF16)
            nc.gpsimd.memset(warm[:], 0.0)
            wpsum = pv_pool.tile([128, 2, VW], F32, tag="pv")
            for _ in range(N_WARMUP):
                nc.tensor.matmul(wpsum[:, 0, 0:32], lhsT=warm[:, 0:128],
                                 rhs=warm[:, 0:32], start=True, stop=True)

            for pair in range(PAIRS):
                # inputs split into pieces ordered by first use so compute
                # starts as soon as the early pieces land
                qt_t, kt_t, v_t = [], [], []

                def load_piece(j, pair=pair, kt_t=kt_t, v_t=v_t):
                    k_tile = kt_pool.tile([128, 512], BF16)
                    nc.sync.dma_start(k_tile[:],
                                      kt[pair, :, j * 512:(j + 1) * 512])
                    kt_t.append(k_tile)
                    vt = v_pool.tile([128, 4, VW], BF16)
                    nc.sync.dma_start(
                        vt[:, :, 0:130],
                        vv[pair, j * 512:(j + 1) * 512, :].rearrange(
                            "(g p) d -> p g d", p=128),
                    )
                    v_t.append(vt)

                def load_q(b, pair=pair, qt_t=qt_t):
                    q_tile = qt_pool.tile([128, 256], BF16)
                    nc.sync.dma_start(q_tile[:],
                                      qt[pair, :, b * 256:(b + 1) * 256])
                    qt_t.append(q_tile)

                load_q(0)
                load_piece(0)
                load_q(1)
                load_q(2)
                load_piece(1)
                load_q(3)
                load_q(4)
                load_piece(2)
                load_q(5)
                load_q(6)
                load_piece(3)
                load_q(7)

                for b in range(NB):
                    cs = chunks_for_block(b)
                    has_b = 2 in cs or 3 in cs

                    stA = stA_pool.tile([128, 4, 256], F32)
                    stB = None
                    if has_b:
                        stB = stB_pool.tile([128, 2, 256], F32, tag="stB")
                    for c in cs:
                        g = 2 * b - 4 + c
                        if c in A_SLOT:
                            dst = stA[:, A_SLOT[c], :]
                        else:
                            dst = stB[:, B_SLOT[c], :]
                        nc.tensor.matmul(
                            dst,
                            lhsT=kt_t[g // 4][:, (g % 4) * 128:(g % 4 + 1) * 128],
                            rhs=qt_t[b][:],
                            start=True, stop=True,
                        )

                    pTA = ptA_pool.tile([128, 4, 256], BF16)
                    stA_f = stA[:].rearrange("p a f -> p (a f)")
                    pTA_f = pTA[:].rearrange("p a f -> p (a f)")
                    if b >= 2:
                        # every A chunk present: one exp + one mask over the
                        # interior [c5h1 c1 c4 c0h0]; the flat ends are the
                        # fully-masked halves and are never read
                        nc.scalar.activation(pTA_f[:, 128:896],
                                             stA_f[:, 128:896], EXP, scale=SCALE)
                        nc.vector.tensor_mul(
                            pTA_f[:, 128:896], pTA_f[:, 128:896],
                            mask_sb[:].rearrange("p a f -> p (a f)")[:, 128:896])
                    else:
                        # only chunks 4 (slot 2) and 5 (slot 0, h1 half) exist
                        nc.scalar.activation(pTA_f[:, 128:256],
                                             stA_f[:, 128:256], EXP, scale=SCALE)
                        nc.vector.tensor_mul(
                            pTA_f[:, 128:256], pTA_f[:, 128:256],
                            mask_sb[:, 0, 128:256])
                        nc.scalar.activation(pTA_f[:, 512:768],
                                             stA_f[:, 512:768], EXP, scale=SCALE)
                        nc.vector.tensor_mul(
                            pTA_f[:, 512:768], pTA_f[:, 512:768],
                            mask_sb[:, 2, :])
                    pTB = None
                    if has_b:
                        pTB = ptB_pool.tile([128, 2, 256], BF16)
                        nc.scalar.activation(pTB[:], stB[:], EXP, scale=SCALE)

                    pv = pv_pool.tile([128, 2, VW], F32, tag="pv")
                    for h in (0, 1):
                        mms = [c for c in (2, 3, 0, 1, 4, 5)
                               if c in cs and (c, h) not in EMPTY_SUBTILES]
                        for i, c in enumerate(mms):
                            g = 2 * b - 4 + c
                            if c in A_SLOT:
                                lhsT = pTA[:, A_SLOT[c], h * 128:(h + 1) * 128]
                            else:
                                lhsT = pTB[:, B_SLOT[c], h * 128:(h + 1) * 128]
                            nc.tensor.matmul(
                                pv[:, h, 0:130],
                                lhsT=lhsT,
                                rhs=v_t[g // 4][:, g % 4, 0:130],
                                start=(i == 0), stop=(i == len(mms) - 1),
                            )

                    recip = rcp_pool.tile([128, 2], F32)
                    nc.vector.reciprocal(recip[:], pv[:, :, 128])
                    ot = out_pool.tile([128, 2, 128], F32)
                    nc.vector.tensor_mul(
                        ot[:], pv[:, :, 0:128],
                        recip[:].unsqueeze(2).broadcast_to([128, 2, 128]),
                    )
                    nc.gpsimd.dma_start(
                        out[pair, b * 256:(b + 1) * 256, :].rearrange(
                            "(h p) d -> p h d", h=2),
                        ot[:],
                    )
    nc.compile()
    return nc


_CACHE: dict = {}


def _get_program() -> bacc.Bacc:
    if "nc" not in _CACHE:
        _CACHE["nc"] = build_program()
    return _CACHE["nc"]


def make_in_maps(query, key, value):
    """Shard + pre-transpose full [B,S,H,D] inputs into per-core input maps."""
    qt_all = query.transpose(0, 2, 3, 1).astype(NP_BF16)   # [B,H,D,S]
    kt_all = key.transpose(0, 2, 3, 1).astype(NP_BF16)
    v_all = np.empty((B, H, S, 130), NP_BF16)              # [B,H,S,D+2ones]
    v_all[:, :, :, 0:128] = value.transpose(0, 2, 1, 3).astype(NP_BF16)
    v_all[:, :, :, 128:130] = 1.0
    masks = build_masks()
    in_maps = []
    for c in range(N_CORES):
        idx = [divmod(c * PAIRS + i, H) for i in range(PAIRS)]
        in_maps.append({
            "qt": np.ascontiguousarray(np.stack([qt_all[b, h] for b, h in idx])),
            "kt": np.ascontiguousarray(np.stack([kt_all[b, h] for b, h in idx])),
            "v": np.ascontiguousarray(np.stack([v_all[b, h] for b, h in idx])),
            "masks": masks,
        })
    return in_maps


def gather_output(results) -> np.ndarray:
    out = np.empty((B, S, H, D), np.float32)
    for c in range(N_CORES):
        o = results[c]["out"]
        for i in range(PAIRS):
            b, h = divmod(c * PAIRS + i, H)
            out[b, :, h, :] = o[i]
    return out


def run(query, key, value, trace: bool = False):
    nc = _get_program()
    in_maps = make_in_maps(query, key, value)
    res = run_bass_kernel_spmd(nc, in_maps, core_ids=list(range(N_CORES)),
                               trace=trace)
    return gather_output(res.results), res


def kernel(query, key, value):
    out, _ = run(query, key, value)
    return out
